# revision 48
# baseline (speedup 1.0000x reference)
"""AttentionTSSA Trainium2 kernel (v2).

Sharding: data-parallel over batch. B=8 -> one batch element per NeuronCore,
zero collectives. Host slices inputs / stacks outputs.

Per-core math (x: [N=4096, D=1024], heads h=16, head dim d=64):
  w[c, n]   = (x @ W_qkv.T).T                 (c = h*64+dd, channel-major)
  s[c]      = sum_n w^2
  logits[h,n] = sum_dd w^2[c,n] * temp[h]/max(s[c],eps)
  Pi        = softmax_h(logits)
  dots[c]   = (sum_n Pi[h(c),n] * w^2[c,n]) / (sum_n Pi[h(c),n] + 1e-8)
  u         = w * Pi_bcast          (overwrites w in place)
  y         = u.T @ (-1/(1+dots) * W_out.T) + b_out

Layout/engine plan (local HW exec ~345us vs ~634us for the v1 kernel):
  - x, W_qkv.T, W_out.T shipped bf16 from host; xT produced by DMA xbar
    transpose (HBM->SBUF, no PE/ACT cost). All MM in bf16 at full PE rate.
  - DMA ordering rule (measured): same-queue, same-type DMAs pipeline at
    ~1.3us cadence; every type (direct2d <-> transpose) or queue switch
    costs a ~3us serializing handoff -> batch by type, one switch each.
  - phase A (PE-bound, 99% busy): MM1; PSUM evicted twice: ACT copy -> w
    (bf16) and DVE STT -> w^2 (bf16) + s accumulator. Stats batched into
    [128, NT]-wide ops; per-head scalars broadcast via tiny sel matmuls.
  - phase D (DVE-bound): per 512-chunk: logits matmul (PE) -> Exp (ACT,
    the only act-table func used) -> sum_h (PE) -> f32r evict (ACT) ->
    bcast (PE) -> reciprocal_approx_fast + Pi STT (DVE); per (tile, 1024-
    chunk): Pi bcast matmul (PE) + ACT evict + DVE dots-STT + DVE u-mult
    (u overwrites w in place; w^2 stays pristine). Last block runs dots
    before u so attn -> wob -> MM2 unblocks early.
  - MM2 (PE-bound, 99% busy): pure PE accumulation, DVE STT eviction with
    fused bias add, per-128-row gpsimd-issued output DMAs.
"""

import sys

sys.path.insert(0, "/opt/trn_rl_repo")

import numpy as np
import concourse.bacc as bacc
import concourse.tile as tile
from concourse import mybir
from concourse.bass_utils import run_bass_kernel_spmd

F32 = mybir.dt.float32
F32R = mybir.dt.float32r
BF16 = mybir.dt.bfloat16
MUL = mybir.AluOpType.mult
ADD = mybir.AluOpType.add
EXP = mybir.ActivationFunctionType.Exp

B, N, D = 8, 4096, 1024
H, HD = 16, 64
P = 128
NT = D // P          # 8 col-partition tiles
CH = 512             # n-chunk for MM work
NCH = N // CH        # 8 chunks
QN = 1024            # n-quarter for xT DMA transposes
NQ = N // QN         # 4 quarters


def build():
    nc = bacc.Bacc()
    x_t = nc.dram_tensor("xTbf", [D, N], BF16, kind="ExternalInput")   # x.T
    wq_t = nc.dram_tensor("wqT", [D, D], BF16, kind="ExternalInput")     # W_qkv.T
    wo_t = nc.dram_tensor("woT", [D, D], BF16, kind="ExternalInput")     # W_out.T
    temp_t = nc.dram_tensor("temp", [H, 1], F32, kind="ExternalInput")
    sel_t = nc.dram_tensor("sel", [NT, H, P], F32, kind="ExternalInput")
    selb_t = nc.dram_tensor("selb", [NT, H, P], BF16, kind="ExternalInput")
    selT_t = nc.dram_tensor("selT", [NT, P, H], F32, kind="ExternalInput")
    bias_t = nc.dram_tensor("bout", [1, D], F32R, kind="ExternalInput")
    y_t = nc.dram_tensor("y", [N, D], F32, kind="ExternalOutput")

    with tile.TileContext(nc) as tc:
        with (
            tc.tile_pool(name="consts", bufs=1) as consts,
            tc.tile_pool(name="wmat", bufs=1) as wmat,
            tc.tile_pool(name="wsb", bufs=1) as wsb,
            tc.tile_pool(name="small", bufs=1) as small,
        ):
            # ---------- weights first (gate MM1 start), then constants ----
            # persistent big tensors: w (becomes u in place), w^2
            w_tiles = [wsb.tile([P, N], BF16, tag=f"w{t}", name=f"w{t}") for t in range(NT)]
            w2_tiles = [wsb.tile([P, N], BF16, tag=f"w2_{t}", name=f"w2_{t}") for t in range(NT)]
            s_all = small.tile([P, NT * NCH], F32, tag="s_all")
            d_all = small.tile([P, NT * (N // 1024)], F32, tag="d_all")
            sumpi_c = small.tile([H, NCH], F32, tag="sumpi_c")
            bias_sb = small.tile([P, D], F32, tag="bias_sb")

            # ---------- phase A: xT by DMA transpose; w = x @ WqkvT ----------
            with (
                tc.tile_pool(name="xq", bufs=2) as xqp,
                tc.tile_pool(name="scrA", bufs=2) as scrA,
                tc.tile_pool(name="psA", bufs=3, space="PSUM") as psA,
                tc.tile_pool(name="psBb", bufs=1, space="PSUM") as psBb,
            ):
                # q0 transposes FIRST in program order: the first DMA issued
                # gets a conservative cross-queue ordering wait, and nothing
                # may gate the transposes that in turn gate MM1
                # DMA ordering rules learned from traces: same-queue, same-
                # TYPE DMAs pipeline at full rate; every transition between
                # DMA types (direct2d <-> xbar transpose) or queues costs a
                # ~3us serializing semaphore handoff. So: one direct2d batch
                # (wq, gates MM1), one transpose batch (q0+q1), then the
                # small constants (needed only ~100us in), then q2/q3.
                wq_sb = wmat.tile([P, NT, D], BF16, tag="wm")
                for k in range(NT):
                    nc.sync.dma_start(
                        out=wq_sb[:, k, :], in_=wq_t[k * P : (k + 1) * P, :]
                    )
                xq01 = []
                for q in range(2):
                    xq = xqp.tile([P, NT, QN], BF16, tag="xq")
                    xq01.append(xq)
                    for k in range(NT):
                        nc.sync.dma_start(
                            out=xq[:, k, :],
                            in_=x_t[k * P : (k + 1) * P, q * QN : (q + 1) * QN],
                        )
                temp_sb = consts.tile([H, 1], F32)
                nc.sync.dma_start(out=temp_sb, in_=temp_t[:, :])
                bias_r = consts.tile([1, D], F32R)
                nc.sync.dma_start(out=bias_r, in_=bias_t[:, :])
                sel_sb = consts.tile([H, NT, P], F32)
                nc.sync.dma_start(out=sel_sb, in_=sel_t.rearrange("t h p -> h t p"))
                selb_sb = consts.tile([H, NT, P], BF16)
                nc.sync.dma_start(out=selb_sb, in_=selb_t.rearrange("t h p -> h t p"))
                selT_sb = consts.tile([P, NT, H], F32)
                nc.sync.dma_start(out=selT_sb, in_=selT_t.rearrange("t p h -> p t h"))
                ones16_f = consts.tile([H, 1], F32)
                nc.vector.memset(ones16_f, 1.0)
                ones16_r = consts.tile([H, 1], F32R)
                nc.vector.tensor_copy(ones16_r, ones16_f)
                ones1x16_f = consts.tile([1, H], F32)
                nc.vector.memset(ones1x16_f, 1.0)
                ones1x16_r = consts.tile([1, H], F32R)
                nc.vector.tensor_copy(ones1x16_r, ones1x16_f)
                ones1x128_f = consts.tile([1, P], F32)
                nc.vector.memset(ones1x128_f, 1.0)
                ones1x128_r = consts.tile([1, P], F32R)
                nc.vector.tensor_copy(ones1x128_r, ones1x128_f)

                for q in range(NQ):
                    if q < 2:
                        xq = xq01[q]
                    else:
                        xq = xqp.tile([P, NT, QN], BF16, tag="xq")
                        for k in range(NT):
                            nc.sync.dma_start(
                                out=xq[:, k, :],
                                in_=x_t[k * P : (k + 1) * P, q * QN : (q + 1) * QN],
                            )
                    for c2 in range(QN // CH):
                        c = q * (QN // CH) + c2
                        cs = slice(c * CH, (c + 1) * CH)
                        for t in range(NT):
                            w_ps = psA.tile([P, CH], F32, tag="mm1")
                            for k in range(NT):
                                nc.tensor.matmul(
                                    w_ps,
                                    wq_sb[:, k, t * P : (t + 1) * P],
                                    xq[:, k, c2 * CH : (c2 + 1) * CH],
                                    start=(k == 0),
                                    stop=(k == NT - 1),
                                )
                            nc.scalar.copy(out=w_tiles[t][:, cs], in_=w_ps)
                            nc.vector.scalar_tensor_tensor(
                                out=w2_tiles[t][:, cs],
                                in0=w_tiles[t][:, cs],
                                scalar=1.0,
                                in1=w_tiles[t][:, cs],
                                op0=MUL,
                                op1=MUL,
                                accum_out=s_all[:, t * NCH + c : t * NCH + c + 1],
                            )

                # bias broadcast [128, D] via 1-row matmuls; emitted last so
                # the in-order PE queue isn't head-of-line blocked waiting
                # for the bias DMA at startup
                bb_ps = psBb.tile([P, D], F32, tag="bb")
                for oh in range(2):
                    os_ = slice(oh * CH, (oh + 1) * CH)
                    nc.tensor.matmul(
                        bb_ps[:, os_], ones1x128_r, bias_r[:, os_],
                        start=True, stop=True,
                    )
                nc.scalar.copy(out=bias_sb, in_=bb_ps)

            # W_out.T load (reuses wq's slot; overlaps phase D)
            wo_sb = wmat.tile([P, NT, D], BF16, tag="wm")
            for k in range(NT):
                nc.sync.dma_start(out=wo_sb[:, k, :], in_=wo_t[k * P : (k + 1) * P, :])

            # ---------- stats 1: lbig[t][c, h] = sel*temp[h]/max(s,eps) ----
            with tc.tile_pool(name="psS1", bufs=1, space="PSUM") as psS1:
                tb_ps = psS1.tile([P, NT], F32, tag="tb")
                for t in range(NT):
                    nc.tensor.matmul(
                        tb_ps[:, t : t + 1], sel_sb[:, t, :], temp_sb,
                        start=True, stop=True,
                    )
                s_red = small.tile([P, NT], F32, tag="s_red")
                nc.vector.reduce_sum(
                    s_red,
                    s_all.rearrange("p (t c) -> p t c", c=NCH),
                    axis=mybir.AxisListType.X,
                )
                nc.vector.tensor_scalar_max(out=s_red, in0=s_red, scalar1=1e-24)
                rcp = small.tile([P, NT], F32, tag="rcp")
                nc.vector.reciprocal(rcp, s_red)
                inv_all = small.tile([P, NT], F32, tag="inv_all")
                nc.vector.tensor_mul(inv_all, rcp, tb_ps)
                lbig = small.tile([P, NT, H], BF16, tag="lbig")
                for t in range(NT):
                    nc.vector.tensor_scalar_mul(
                        out=lbig[:, t, :],
                        in0=selT_sb[:, t, :],
                        scalar1=inv_all[:, t : t + 1],
                    )

            # ---------- phase D: logits -> softmax -> u, dots (pipelined) --
            # logits/softmax work per 512-chunk (PSUM-bank sized); the bulk
            # pib/u/dots passes run 1024-wide to halve DVE/ACT op count.
            pi_bf = small.tile([H, N], BF16, tag="pi_bf")
            CHD = 1024
            NCHD = N // CHD
            with (
                tc.tile_pool(name="scrD", bufs=3) as scrD,
                tc.tile_pool(name="psL", bufs=2, space="PSUM") as psL,
                tc.tile_pool(name="psS", bufs=1, space="PSUM") as psS,
                tc.tile_pool(name="psP", bufs=2, space="PSUM") as psP,
            ):
                for cd in range(NCHD):
                    for hh in range(CHD // CH):
                        c = cd * (CHD // CH) + hh
                        cs = slice(c * CH, (c + 1) * CH)
                        lg_ps = psL.tile([H, CH], F32, tag="lg")
                        for t in range(NT):
                            nc.tensor.matmul(
                                lg_ps, lbig[:, t, :], w2_tiles[t][:, cs],
                                start=(t == 0), stop=(t == NT - 1),
                            )
                        e_sb = scrD.tile([H, CH], F32R, tag="e_sb")
                        nc.scalar.activation(out=e_sb, in_=lg_ps, func=EXP)
                        se_ps = psS.tile([1, CH], F32, tag="se")
                        nc.tensor.matmul(
                            se_ps, ones16_r, e_sb, start=True, stop=True
                        )
                        ses = scrD.tile([1, CH], F32R, tag="ses", bufs=2)
                        nc.scalar.copy(out=ses, in_=se_ps)
                        rb_ps = psS.tile([H, CH], F32, tag="rb")
                        nc.tensor.matmul(
                            rb_ps, ones1x16_r, ses, start=True, stop=True
                        )
                        rcb = scrD.tile([H, CH], F32, tag="rcb", bufs=2)
                        nc.vector.reciprocal_approx_fast(out=rcb, in_=rb_ps)
                        nc.vector.scalar_tensor_tensor(
                            out=pi_bf[:, cs],
                            in0=e_sb.bitcast(F32),
                            scalar=1.0,
                            in1=rcb,
                            op0=MUL,
                            op1=MUL,
                            accum_out=sumpi_c[:, c : c + 1],
                        )
                    ds = slice(cd * CHD, (cd + 1) * CHD)
                    # In the last block, run all dots accumulations before
                    # the u-mults: dots gates attn -> wob -> ALL of MM2,
                    # while the last u-mults only gate MM2's final n-tiles
                    # and hide under its first matmuls.
                    last = cd == NCHD - 1
                    deferred_u = []
                    for t in range(NT):
                        pib_ps = psP.tile([P, CHD], F32, tag="pib")
                        for hh in range(CHD // CH):
                            nc.tensor.matmul(
                                pib_ps[:, hh * CH : (hh + 1) * CH],
                                selb_sb[:, t, :],
                                pi_bf[:, cd * CHD + hh * CH : cd * CHD + (hh + 1) * CH],
                                start=True,
                                stop=True,
                            )
                        if last:
                            pib_sb = scrD.tile(
                                [P, CHD], BF16, tag="pib_last", bufs=NT
                            )
                        else:
                            pib_sb = scrD.tile(
                                [P, CHD], BF16, tag="pib_sb", bufs=2
                            )
                        nc.scalar.copy(out=pib_sb, in_=pib_ps)
                        junk = scrD.tile([P, CHD], BF16, tag="junkD", bufs=1)
                        nc.vector.scalar_tensor_tensor(
                            out=junk,
                            in0=pib_sb,
                            scalar=1.0,
                            in1=w2_tiles[t][:, ds],
                            op0=MUL,
                            op1=MUL,
                            accum_out=d_all[:, t * NCHD + cd : t * NCHD + cd + 1],
                        )

                        if last:
                            deferred_u.append((t, pib_sb))
                        else:
                            # u overwrites w in place (w^2 stays pristine)
                            nc.vector.tensor_mul(
                                w_tiles[t][:, ds], w_tiles[t][:, ds], pib_sb
                            )
                    for t, pib_sb in deferred_u:
                        nc.vector.tensor_mul(
                            w_tiles[t][:, ds], w_tiles[t][:, ds], pib_sb
                        )

            # ---------- stats 2: attn; wob = -attn * WoutT (bf16) ----------
            wob, _wob_free = tc.tile([P, NT, D], BF16, name="wob")
            with tc.tile_pool(name="psS2", bufs=1, space="PSUM") as psS2:
                sumpi = small.tile([H, 1], F32, tag="sumpi")
                nc.vector.reduce_sum(sumpi, sumpi_c, axis=mybir.AxisListType.X)
                nc.vector.tensor_scalar_add(out=sumpi, in0=sumpi, scalar1=1e-8)
                ispi = small.tile([H, 1], F32, tag="ispi")
                nc.vector.reciprocal(ispi, sumpi)
                isp_ps = psS2.tile([P, NT], F32, tag="isp")
                for t in range(NT):
                    nc.tensor.matmul(
                        isp_ps[:, t : t + 1], sel_sb[:, t, :], ispi,
                        start=True, stop=True,
                    )
                d_red = small.tile([P, NT], F32, tag="d_red")
                nc.vector.reduce_sum(
                    d_red,
                    d_all.rearrange("p (t c) -> p t c", c=N // 1024),
                    axis=mybir.AxisListType.X,
                )
                attn_neg = small.tile([P, NT], F32, tag="attn_neg")
                nc.vector.tensor_mul(attn_neg, d_red, isp_ps)
                nc.vector.tensor_scalar_add(out=attn_neg, in0=attn_neg, scalar1=1.0)
                nc.vector.reciprocal(attn_neg, attn_neg)
                nc.vector.tensor_scalar_mul(
                    out=attn_neg, in0=attn_neg, scalar1=-1.0
                )
                for t in range(NT):
                    if t % 2 == 0:
                        nc.scalar.mul(
                            out=wob[:, t, :],
                            in_=wo_sb[:, t, :],
                            mul=attn_neg[:, t : t + 1],
                        )
                    else:
                        nc.vector.tensor_scalar_mul(
                            out=wob[:, t, :],
                            in0=wo_sb[:, t, :],
                            scalar1=attn_neg[:, t : t + 1],
                        )

            # ---------- MM2: y = u.T @ wob + b ----------
            with (
                tc.tile_pool(name="och", bufs=3) as och,
                tc.tile_pool(name="psMM2", bufs=4, space="PSUM") as psMM2,
            ):
                MS = CH // P  # 4 n-subtiles per 512-chunk
                for c in range(NCH):
                    for m in range(MS):
                        ms_ = slice(c * CH + m * P, c * CH + (m + 1) * P)
                        outf = och.tile([P, D], F32, tag="outf")
                        for oh in range(2):
                            os_ = slice(oh * CH, (oh + 1) * CH)
                            f_ps = psMM2.tile([P, CH], F32, tag="mm2")
                            for t in range(NT):
                                nc.tensor.matmul(
                                    f_ps,
                                    w_tiles[t][:, ms_],
                                    wob[:, t, os_],
                                    start=(t == 0),
                                    stop=(t == NT - 1),
                                )
                            nc.vector.scalar_tensor_tensor(
                                out=outf[:, os_],
                                in0=f_ps,
                                scalar=1.0,
                                in1=bias_sb[:, os_],
                                op0=MUL,
                                op1=ADD,
                            )
                        nc.gpsimd.dma_start(out=y_t[ms_, :], in_=outf)
            _wob_free()

    if not nc.is_finalized():
        nc.finalize()
    return nc


_NC_CACHE = None
_LAST_IN_MAPS = None
_RUNNER = None


def _make_runner(nc, n_cores):
    """Like bass2jax.run_bass_via_pjrt but with the jitted callable cached,
    so repeat calls don't re-trace/re-compile the XLA wrapper."""
    import jax
    from jax.experimental.shard_map import shard_map
    from jax.sharding import Mesh, PartitionSpec
    from concourse import mybir as _mybir
    from concourse.bass2jax import (
        _bass_exec_p,
        install_neuronx_cc_hook,
        partition_id_tensor,
    )

    install_neuronx_cc_hook()

    partition_name = nc.partition_id_tensor.name if nc.partition_id_tensor else None
    in_names, out_names, out_avals, zero_outs = [], [], [], []
    for alloc in nc.m.functions[0].allocations:
        if not isinstance(alloc, _mybir.MemoryLocationSet):
            continue
        name = alloc.memorylocations[0].name
        if alloc.kind == "ExternalInput":
            if name != partition_name:
                in_names.append(name)
        elif alloc.kind == "ExternalOutput":
            shape = tuple(alloc.tensor_shape)
            dtype = _mybir.dt.np(alloc.dtype)
            out_names.append(name)
            out_avals.append(jax.core.ShapedArray(shape, dtype))
            zero_outs.append(np.zeros(shape, dtype))
    n_params = len(in_names)
    n_outs = len(out_names)
    all_in_names = in_names + out_names + (
        [partition_name] if partition_name else []
    )
    donate = tuple(range(n_params, n_params + n_outs))

    def _body(*args):
        operands = list(args)
        if partition_name is not None:
            operands.append(partition_id_tensor())
        outs = _bass_exec_p.bind(
            *operands,
            out_avals=tuple(out_avals),
            in_names=tuple(all_in_names),
            out_names=tuple(out_names),
            lowering_input_output_aliases=(),
            sim_require_finite=True,
            sim_require_nnan=True,
            nc=nc,
        )
        return tuple(outs)

    devices = jax.devices()[:n_cores]
    mesh = Mesh(np.asarray(devices), ("core",))
    in_specs = (PartitionSpec("core"),) * (n_params + n_outs)
    out_specs = (PartitionSpec("core"),) * n_outs
    sharded = jax.jit(
        shard_map(
            _body, mesh=mesh, in_specs=in_specs, out_specs=out_specs, check_rep=False
        ),
        donate_argnums=donate,
        keep_unused=True,
    )

    def run(in_maps):
        concat_in = [
            np.concatenate([np.asarray(m[name]) for m in in_maps], axis=0)
            for name in in_names
        ]
        concat_zeros = [
            np.zeros((n_cores * z.shape[0], *z.shape[1:]), z.dtype)
            for z in zero_outs
        ]
        out_arrs = sharded(*concat_in, *concat_zeros)
        return {
            name: np.asarray(out_arrs[i]).reshape(n_cores, *out_avals[i].shape)
            for i, name in enumerate(out_names)
        }

    run.sharded = sharded
    run.meta = (in_names, out_names, out_avals, n_params, n_outs)
    return run


def kernel(x, W_qkv, temp, W_out, b_out):
    global _NC_CACHE, _RUNNER
    if _NC_CACHE is None:
        _NC_CACHE = build()
        _RUNNER = _make_runner(_NC_CACHE, B)

    import ml_dtypes

    bf16 = ml_dtypes.bfloat16
    x = np.asarray(x, dtype=np.float32)
    xbf = x.astype(bf16)
    wqT = np.ascontiguousarray(np.asarray(W_qkv, dtype=np.float32).T).astype(bf16)
    woT = np.ascontiguousarray(np.asarray(W_out, dtype=np.float32).T).astype(bf16)
    temp = np.ascontiguousarray(np.asarray(temp, dtype=np.float32).reshape(H, 1))
    bout = np.ascontiguousarray(np.asarray(b_out, dtype=np.float32).reshape(1, D))

    sel = np.zeros((NT, H, P), dtype=np.float32)
    for t in range(NT):
        sel[t, 2 * t, 0:HD] = 1.0
        sel[t, 2 * t + 1, HD:P] = 1.0
    selT = np.ascontiguousarray(sel.transpose(0, 2, 1))

    in_maps = [
        {"xTbf": np.ascontiguousarray(xbf[i].T), "wqT": wqT, "woT": woT,
         "temp": temp, "bout": bout, "sel": sel, "selb": sel.astype(bf16),
         "selT": selT}
        for i in range(B)
    ]
    global _LAST_IN_MAPS
    _LAST_IN_MAPS = in_maps
    out = _RUNNER(in_maps)
    return out["y"]


if __name__ == "__main__":
    rng = np.random.default_rng(0)
    x = rng.standard_normal((B, N, D), dtype=np.float32)
    W_qkv = (rng.standard_normal((D, D), dtype=np.float32) * 0.02).astype(np.float32)
    temp = np.ones((H, 1), dtype=np.float32)
    W_out = (rng.standard_normal((D, D), dtype=np.float32) * 0.02).astype(np.float32)
    b_out = np.zeros((D,), dtype=np.float32)
    y = kernel(x=x, W_qkv=W_qkv, temp=temp, W_out=W_out, b_out=b_out)
    print("kernel ran, y shape", y.shape, "mean abs", np.abs(y).mean())


# revision 50
# speedup vs baseline: 1.1937x; 1.1937x over previous
"""AttentionTSSA Trainium2 kernel (v2).

Sharding: data-parallel over batch. B=8 -> one batch element per NeuronCore,
zero collectives. Host slices inputs / stacks outputs.

Per-core math (x: [N=4096, D=1024], heads h=16, head dim d=64):
  w[c, n]   = (x @ W_qkv.T).T                 (c = h*64+dd, channel-major)
  s[c]      = sum_n w^2
  logits[h,n] = sum_dd w^2[c,n] * temp[h]/max(s[c],eps)
  Pi        = softmax_h(logits)
  dots[c]   = (sum_n Pi[h(c),n] * w^2[c,n]) / (sum_n Pi[h(c),n] + 1e-8)
  u         = w * Pi_bcast          (overwrites w in place)
  y         = u.T @ (-1/(1+dots) * W_out.T) + b_out

Layout/engine plan (local HW exec ~345us vs ~634us for the v1 kernel):
  - x, W_qkv.T, W_out.T shipped bf16 from host; xT produced by DMA xbar
    transpose (HBM->SBUF, no PE/ACT cost). All MM in bf16 at full PE rate.
  - DMA ordering rule (measured): same-queue, same-type DMAs pipeline at
    ~1.3us cadence; every type (direct2d <-> transpose) or queue switch
    costs a ~3us serializing handoff -> batch by type, one switch each.
  - phase A (PE-bound, 99% busy): MM1; PSUM evicted twice: ACT copy -> w
    (bf16) and DVE STT -> w^2 (bf16) + s accumulator. Stats batched into
    [128, NT]-wide ops; per-head scalars broadcast via tiny sel matmuls.
  - phase D (DVE-bound): per 512-chunk: logits matmul (PE) -> Exp (ACT,
    the only act-table func used) -> sum_h (PE) -> f32r evict (ACT) ->
    bcast (PE) -> reciprocal_approx_fast + Pi STT (DVE); per (tile, 1024-
    chunk): Pi bcast matmul (PE) + ACT evict + DVE dots-STT + DVE u-mult
    (u overwrites w in place; w^2 stays pristine). Last block runs dots
    before u so attn -> wob -> MM2 unblocks early.
  - MM2 (PE-bound, 99% busy): pure PE accumulation, DVE STT eviction with
    fused bias add, per-128-row gpsimd-issued output DMAs.
"""

import sys

sys.path.insert(0, "/opt/trn_rl_repo")

import numpy as np
import concourse.bacc as bacc
import concourse.tile as tile
from concourse import mybir
from concourse.bass_utils import run_bass_kernel_spmd

F32 = mybir.dt.float32
F32R = mybir.dt.float32r
BF16 = mybir.dt.bfloat16
MUL = mybir.AluOpType.mult
ADD = mybir.AluOpType.add
EXP = mybir.ActivationFunctionType.Exp

B, N, D = 8, 4096, 1024
H, HD = 16, 64
P = 128
NT = D // P          # 8 col-partition tiles
CH = 512             # n-chunk for MM work
NCH = N // CH        # 8 chunks
QN = 1024            # n-quarter for xT DMA transposes
NQ = N // QN         # 4 quarters


def build():
    nc = bacc.Bacc()
    x_t = nc.dram_tensor("xTbf", [D, N], BF16, kind="ExternalInput")   # x.T
    wq_t = nc.dram_tensor("wqT", [D, D], BF16, kind="ExternalInput")     # W_qkv.T
    wo_t = nc.dram_tensor("woT", [D, D], BF16, kind="ExternalInput")     # W_out.T
    temp_t = nc.dram_tensor("temp", [H, 1], F32, kind="ExternalInput")
    sel_t = nc.dram_tensor("sel", [NT, H, P], F32, kind="ExternalInput")
    selb_t = nc.dram_tensor("selb", [NT, H, P], BF16, kind="ExternalInput")
    selT_t = nc.dram_tensor("selT", [NT, P, H], F32, kind="ExternalInput")
    bias_t = nc.dram_tensor("bout", [1, D], F32R, kind="ExternalInput")
    y_t = nc.dram_tensor("y", [N, D], F32, kind="ExternalOutput")

    with tile.TileContext(nc) as tc:
        with (
            tc.tile_pool(name="consts", bufs=1) as consts,
            tc.tile_pool(name="wmat", bufs=1) as wmat,
            tc.tile_pool(name="wsb", bufs=1) as wsb,
            tc.tile_pool(name="small", bufs=1) as small,
        ):
            # ---------- weights first (gate MM1 start), then constants ----
            # persistent big tensors: w (becomes u in place), w^2
            w_tiles = [wsb.tile([P, N], BF16, tag=f"w{t}", name=f"w{t}") for t in range(NT)]
            w2_tiles = [wsb.tile([P, N], BF16, tag=f"w2_{t}", name=f"w2_{t}") for t in range(NT)]
            s_all = small.tile([P, NT * NCH], F32, tag="s_all")
            d_all = small.tile([P, NT * (N // 1024)], F32, tag="d_all")
            sumpi_c = small.tile([H, NCH], F32, tag="sumpi_c")
            bias_sb = small.tile([P, D], F32, tag="bias_sb")

            # ---------- phase A: xT by DMA transpose; w = x @ WqkvT ----------
            with (
                tc.tile_pool(name="xq", bufs=2) as xqp,
                tc.tile_pool(name="scrA", bufs=2) as scrA,
                tc.tile_pool(name="psA", bufs=3, space="PSUM") as psA,
                tc.tile_pool(name="psBb", bufs=1, space="PSUM") as psBb,
            ):
                # q0 transposes FIRST in program order: the first DMA issued
                # gets a conservative cross-queue ordering wait, and nothing
                # may gate the transposes that in turn gate MM1
                # DMA ordering rules learned from traces: same-queue, same-
                # TYPE DMAs pipeline at full rate; every transition between
                # DMA types (direct2d <-> xbar transpose) or queues costs a
                # ~3us serializing semaphore handoff. So: one direct2d batch
                # (wq, gates MM1), one transpose batch (q0+q1), then the
                # small constants (needed only ~100us in), then q2/q3.
                wq_sb = wmat.tile([P, NT, D], BF16, tag="wm")
                for k in range(NT):
                    nc.sync.dma_start(
                        out=wq_sb[:, k, :], in_=wq_t[k * P : (k + 1) * P, :]
                    )
                xq01 = []
                for q in range(2):
                    xq = xqp.tile([P, NT, QN], BF16, tag="xq")
                    xq01.append(xq)
                    for k in range(NT):
                        nc.sync.dma_start(
                            out=xq[:, k, :],
                            in_=x_t[k * P : (k + 1) * P, q * QN : (q + 1) * QN],
                        )
                temp_sb = consts.tile([H, 1], F32)
                nc.sync.dma_start(out=temp_sb, in_=temp_t[:, :])
                bias_r = consts.tile([1, D], F32R)
                nc.sync.dma_start(out=bias_r, in_=bias_t[:, :])
                sel_sb = consts.tile([H, NT, P], F32)
                nc.sync.dma_start(out=sel_sb, in_=sel_t.rearrange("t h p -> h t p"))
                selb_sb = consts.tile([H, NT, P], BF16)
                nc.sync.dma_start(out=selb_sb, in_=selb_t.rearrange("t h p -> h t p"))
                selT_sb = consts.tile([P, NT, H], F32)
                nc.sync.dma_start(out=selT_sb, in_=selT_t.rearrange("t p h -> p t h"))
                ones16_f = consts.tile([H, 1], F32)
                nc.vector.memset(ones16_f, 1.0)
                ones16_r = consts.tile([H, 1], F32R)
                nc.vector.tensor_copy(ones16_r, ones16_f)
                ones1x16_f = consts.tile([1, H], F32)
                nc.vector.memset(ones1x16_f, 1.0)
                ones1x16_r = consts.tile([1, H], F32R)
                nc.vector.tensor_copy(ones1x16_r, ones1x16_f)
                ones1x128_f = consts.tile([1, P], F32)
                nc.vector.memset(ones1x128_f, 1.0)
                ones1x128_r = consts.tile([1, P], F32R)
                nc.vector.tensor_copy(ones1x128_r, ones1x128_f)

                for q in range(NQ):
                    if q < 2:
                        xq = xq01[q]
                    else:
                        xq = xqp.tile([P, NT, QN], BF16, tag="xq")
                        for k in range(NT):
                            nc.sync.dma_start(
                                out=xq[:, k, :],
                                in_=x_t[k * P : (k + 1) * P, q * QN : (q + 1) * QN],
                            )
                    for c2 in range(QN // CH):
                        c = q * (QN // CH) + c2
                        cs = slice(c * CH, (c + 1) * CH)
                        for t in range(NT):
                            w_ps = psA.tile([P, CH], F32, tag="mm1")
                            for k in range(NT):
                                nc.tensor.matmul(
                                    w_ps,
                                    wq_sb[:, k, t * P : (t + 1) * P],
                                    xq[:, k, c2 * CH : (c2 + 1) * CH],
                                    start=(k == 0),
                                    stop=(k == NT - 1),
                                )
                            nc.scalar.copy(out=w_tiles[t][:, cs], in_=w_ps)
                            nc.vector.scalar_tensor_tensor(
                                out=w2_tiles[t][:, cs],
                                in0=w_tiles[t][:, cs],
                                scalar=1.0,
                                in1=w_tiles[t][:, cs],
                                op0=MUL,
                                op1=MUL,
                                accum_out=s_all[:, t * NCH + c : t * NCH + c + 1],
                            )

                # bias broadcast [128, D] via 1-row matmuls; emitted last so
                # the in-order PE queue isn't head-of-line blocked waiting
                # for the bias DMA at startup
                bb_ps = psBb.tile([P, D], F32, tag="bb")
                for oh in range(2):
                    os_ = slice(oh * CH, (oh + 1) * CH)
                    nc.tensor.matmul(
                        bb_ps[:, os_], ones1x128_r, bias_r[:, os_],
                        start=True, stop=True,
                    )
                nc.scalar.copy(out=bias_sb, in_=bb_ps)

            # W_out.T load (reuses wq's slot; overlaps phase D)
            wo_sb = wmat.tile([P, NT, D], BF16, tag="wm")
            for k in range(NT):
                nc.sync.dma_start(out=wo_sb[:, k, :], in_=wo_t[k * P : (k + 1) * P, :])

            # ---------- stats 1: lbig[t][c, h] = sel*temp[h]/max(s,eps) ----
            with tc.tile_pool(name="psS1", bufs=1, space="PSUM") as psS1:
                tb_ps = psS1.tile([P, NT], F32, tag="tb")
                for t in range(NT):
                    nc.tensor.matmul(
                        tb_ps[:, t : t + 1], sel_sb[:, t, :], temp_sb,
                        start=True, stop=True,
                    )
                s_red = small.tile([P, NT], F32, tag="s_red")
                nc.vector.reduce_sum(
                    s_red,
                    s_all.rearrange("p (t c) -> p t c", c=NCH),
                    axis=mybir.AxisListType.X,
                )
                nc.vector.tensor_scalar_max(out=s_red, in0=s_red, scalar1=1e-24)
                rcp = small.tile([P, NT], F32, tag="rcp")
                nc.vector.reciprocal(rcp, s_red)
                inv_all = small.tile([P, NT], F32, tag="inv_all")
                nc.vector.tensor_mul(inv_all, rcp, tb_ps)
                lbig = small.tile([P, NT, H], BF16, tag="lbig")
                for t in range(NT):
                    nc.vector.tensor_scalar_mul(
                        out=lbig[:, t, :],
                        in0=selT_sb[:, t, :],
                        scalar1=inv_all[:, t : t + 1],
                    )

            # ---------- phase D: logits -> softmax -> u, dots (pipelined) --
            # logits/softmax work per 512-chunk (PSUM-bank sized); the bulk
            # pib/u/dots passes run 1024-wide to halve DVE/ACT op count.
            pi_bf = small.tile([H, N], BF16, tag="pi_bf")
            CHD = 1024
            NCHD = N // CHD
            with (
                tc.tile_pool(name="scrD", bufs=3) as scrD,
                tc.tile_pool(name="psL", bufs=2, space="PSUM") as psL,
                tc.tile_pool(name="psS", bufs=1, space="PSUM") as psS,
                tc.tile_pool(name="psP", bufs=2, space="PSUM") as psP,
            ):
                for cd in range(NCHD):
                    for hh in range(CHD // CH):
                        c = cd * (CHD // CH) + hh
                        cs = slice(c * CH, (c + 1) * CH)
                        lg_ps = psL.tile([H, CH], F32, tag="lg")
                        for t in range(NT):
                            nc.tensor.matmul(
                                lg_ps, lbig[:, t, :], w2_tiles[t][:, cs],
                                start=(t == 0), stop=(t == NT - 1),
                            )
                        e_sb = scrD.tile([H, CH], F32R, tag="e_sb")
                        nc.scalar.activation(out=e_sb, in_=lg_ps, func=EXP)
                        se_ps = psS.tile([1, CH], F32, tag="se")
                        nc.tensor.matmul(
                            se_ps, ones16_r, e_sb, start=True, stop=True
                        )
                        ses = scrD.tile([1, CH], F32R, tag="ses", bufs=2)
                        nc.scalar.copy(out=ses, in_=se_ps)
                        rb_ps = psS.tile([H, CH], F32, tag="rb")
                        nc.tensor.matmul(
                            rb_ps, ones1x16_r, ses, start=True, stop=True
                        )
                        rcb = scrD.tile([H, CH], F32, tag="rcb", bufs=2)
                        nc.vector.reciprocal_approx_fast(out=rcb, in_=rb_ps)
                        nc.vector.scalar_tensor_tensor(
                            out=pi_bf[:, cs],
                            in0=e_sb.bitcast(F32),
                            scalar=1.0,
                            in1=rcb,
                            op0=MUL,
                            op1=MUL,
                            accum_out=sumpi_c[:, c : c + 1],
                        )
                    ds = slice(cd * CHD, (cd + 1) * CHD)
                    # In the last block, run all dots accumulations before
                    # the u-mults: dots gates attn -> wob -> ALL of MM2,
                    # while the last u-mults only gate MM2's final n-tiles
                    # and hide under its first matmuls.
                    last = cd == NCHD - 1
                    deferred_u = []
                    for t in range(NT):
                        pib_ps = psP.tile([P, CHD], F32, tag="pib")
                        for hh in range(CHD // CH):
                            nc.tensor.matmul(
                                pib_ps[:, hh * CH : (hh + 1) * CH],
                                selb_sb[:, t, :],
                                pi_bf[:, cd * CHD + hh * CH : cd * CHD + (hh + 1) * CH],
                                start=True,
                                stop=True,
                            )
                        if last:
                            pib_sb = scrD.tile(
                                [P, CHD], BF16, tag="pib_last", bufs=NT
                            )
                        else:
                            pib_sb = scrD.tile(
                                [P, CHD], BF16, tag="pib_sb", bufs=2
                            )
                        nc.scalar.copy(out=pib_sb, in_=pib_ps)
                        junk = scrD.tile([P, CHD], BF16, tag="junkD", bufs=1)
                        nc.vector.scalar_tensor_tensor(
                            out=junk,
                            in0=pib_sb,
                            scalar=1.0,
                            in1=w2_tiles[t][:, ds],
                            op0=MUL,
                            op1=MUL,
                            accum_out=d_all[:, t * NCHD + cd : t * NCHD + cd + 1],
                        )

                        if last:
                            deferred_u.append((t, pib_sb))
                        else:
                            # u overwrites w in place (w^2 stays pristine)
                            nc.vector.tensor_mul(
                                w_tiles[t][:, ds], w_tiles[t][:, ds], pib_sb
                            )
                    for t, pib_sb in deferred_u:
                        nc.vector.tensor_mul(
                            w_tiles[t][:, ds], w_tiles[t][:, ds], pib_sb
                        )

            # ---------- stats 2: attn; wob = -attn * WoutT (bf16) ----------
            wob, _wob_free = tc.tile([P, NT, D], BF16, name="wob")
            with tc.tile_pool(name="psS2", bufs=1, space="PSUM") as psS2:
                sumpi = small.tile([H, 1], F32, tag="sumpi")
                nc.vector.reduce_sum(sumpi, sumpi_c, axis=mybir.AxisListType.X)
                nc.vector.tensor_scalar_add(out=sumpi, in0=sumpi, scalar1=1e-8)
                ispi = small.tile([H, 1], F32, tag="ispi")
                nc.vector.reciprocal(ispi, sumpi)
                isp_ps = psS2.tile([P, NT], F32, tag="isp")
                for t in range(NT):
                    nc.tensor.matmul(
                        isp_ps[:, t : t + 1], sel_sb[:, t, :], ispi,
                        start=True, stop=True,
                    )
                d_red = small.tile([P, NT], F32, tag="d_red")
                nc.vector.reduce_sum(
                    d_red,
                    d_all.rearrange("p (t c) -> p t c", c=N // 1024),
                    axis=mybir.AxisListType.X,
                )
                attn_neg = small.tile([P, NT], F32, tag="attn_neg")
                nc.vector.tensor_mul(attn_neg, d_red, isp_ps)
                nc.vector.tensor_scalar_add(out=attn_neg, in0=attn_neg, scalar1=1.0)
                nc.vector.reciprocal(attn_neg, attn_neg)
                nc.vector.tensor_scalar_mul(
                    out=attn_neg, in0=attn_neg, scalar1=-1.0
                )
                for t in range(NT):
                    if t % 2 == 0:
                        nc.scalar.mul(
                            out=wob[:, t, :],
                            in_=wo_sb[:, t, :],
                            mul=attn_neg[:, t : t + 1],
                        )
                    else:
                        nc.vector.tensor_scalar_mul(
                            out=wob[:, t, :],
                            in0=wo_sb[:, t, :],
                            scalar1=attn_neg[:, t : t + 1],
                        )

            # ---------- MM2: y = u.T @ wob + b ----------
            with (
                tc.tile_pool(name="och", bufs=3) as och,
                tc.tile_pool(name="psMM2", bufs=4, space="PSUM") as psMM2,
            ):
                MS = CH // P  # 4 n-subtiles per 512-chunk
                for c in range(NCH):
                    for m in range(MS):
                        ms_ = slice(c * CH + m * P, c * CH + (m + 1) * P)
                        outf = och.tile([P, D], F32, tag="outf")
                        for oh in range(2):
                            os_ = slice(oh * CH, (oh + 1) * CH)
                            f_ps = psMM2.tile([P, CH], F32, tag="mm2")
                            for t in range(NT):
                                nc.tensor.matmul(
                                    f_ps,
                                    w_tiles[t][:, ms_],
                                    wob[:, t, os_],
                                    start=(t == 0),
                                    stop=(t == NT - 1),
                                )
                            nc.vector.scalar_tensor_tensor(
                                out=outf[:, os_],
                                in0=f_ps,
                                scalar=1.0,
                                in1=bias_sb[:, os_],
                                op0=MUL,
                                op1=ADD,
                            )
                        nc.gpsimd.dma_start(out=y_t[ms_, :], in_=outf)
            _wob_free()

    if not nc.is_finalized():
        nc.finalize()
    return nc


_NC_CACHE = None
_LAST_IN_MAPS = None
_RUNNER = None


def _make_runner(nc, n_cores):
    """Like bass2jax.run_bass_via_pjrt but with the jitted callable cached,
    so repeat calls don't re-trace/re-compile the XLA wrapper."""
    import jax
    from jax.experimental.shard_map import shard_map
    from jax.sharding import Mesh, PartitionSpec
    from concourse import mybir as _mybir
    from concourse.bass2jax import (
        _bass_exec_p,
        install_neuronx_cc_hook,
        partition_id_tensor,
    )

    install_neuronx_cc_hook()

    partition_name = nc.partition_id_tensor.name if nc.partition_id_tensor else None
    in_names, out_names, out_avals, zero_outs = [], [], [], []
    for alloc in nc.m.functions[0].allocations:
        if not isinstance(alloc, _mybir.MemoryLocationSet):
            continue
        name = alloc.memorylocations[0].name
        if alloc.kind == "ExternalInput":
            if name != partition_name:
                in_names.append(name)
        elif alloc.kind == "ExternalOutput":
            shape = tuple(alloc.tensor_shape)
            dtype = _mybir.dt.np(alloc.dtype)
            out_names.append(name)
            out_avals.append(jax.core.ShapedArray(shape, dtype))
            zero_outs.append(np.zeros(shape, dtype))
    n_params = len(in_names)
    n_outs = len(out_names)
    all_in_names = in_names + out_names + (
        [partition_name] if partition_name else []
    )
    donate = tuple(range(n_params, n_params + n_outs))

    def _body(*args):
        operands = list(args)
        if partition_name is not None:
            operands.append(partition_id_tensor())
        outs = _bass_exec_p.bind(
            *operands,
            out_avals=tuple(out_avals),
            in_names=tuple(all_in_names),
            out_names=tuple(out_names),
            lowering_input_output_aliases=(),
            sim_require_finite=True,
            sim_require_nnan=True,
            nc=nc,
        )
        return tuple(outs)

    devices = jax.devices()[:n_cores]
    mesh = Mesh(np.asarray(devices), ("core",))
    in_specs = (PartitionSpec("core"),) * (n_params + n_outs)
    out_specs = (PartitionSpec("core"),) * n_outs
    sharded = jax.jit(
        shard_map(
            _body, mesh=mesh, in_specs=in_specs, out_specs=out_specs, check_rep=False
        ),
        donate_argnums=donate,
        keep_unused=True,
    )

    def run(in_maps):
        concat_in = [
            np.concatenate([np.asarray(m[name]) for m in in_maps], axis=0)
            for name in in_names
        ]
        concat_zeros = [
            np.zeros((n_cores * z.shape[0], *z.shape[1:]), z.dtype)
            for z in zero_outs
        ]
        out_arrs = sharded(*concat_in, *concat_zeros)
        return {
            name: np.asarray(out_arrs[i]).reshape(n_cores, *out_avals[i].shape)
            for i, name in enumerate(out_names)
        }

    run.sharded = sharded
    run.meta = (in_names, out_names, out_avals, n_params, n_outs)
    return run


def kernel(x, W_qkv, temp, W_out, b_out):
    global _NC_CACHE, _RUNNER
    if _NC_CACHE is None:
        _NC_CACHE = build()
        _RUNNER = _make_runner(_NC_CACHE, B)

    import ml_dtypes

    bf16 = ml_dtypes.bfloat16
    x = np.asarray(x, dtype=np.float32)
    xbf = x.astype(bf16)
    wqT = np.ascontiguousarray(np.asarray(W_qkv, dtype=np.float32).T).astype(bf16)
    woT = np.ascontiguousarray(np.asarray(W_out, dtype=np.float32).T).astype(bf16)
    temp = np.ascontiguousarray(np.asarray(temp, dtype=np.float32).reshape(H, 1))
    bout = np.ascontiguousarray(np.asarray(b_out, dtype=np.float32).reshape(1, D))

    sel = np.zeros((NT, H, P), dtype=np.float32)
    for t in range(NT):
        sel[t, 2 * t, 0:HD] = 1.0
        sel[t, 2 * t + 1, HD:P] = 1.0
    selT = np.ascontiguousarray(sel.transpose(0, 2, 1))

    in_maps = [
        {"xTbf": np.ascontiguousarray(xbf[i].T), "wqT": wqT, "woT": woT,
         "temp": temp, "bout": bout, "sel": sel, "selb": sel.astype(bf16),
         "selT": selT}
        for i in range(B)
    ]
    global _LAST_IN_MAPS
    _LAST_IN_MAPS = in_maps
    out = _RUNNER(in_maps)
    return out["y"]


if __name__ == "__main__":
    rng = np.random.default_rng(0)
    x = rng.standard_normal((B, N, D), dtype=np.float32)
    W_qkv = (rng.standard_normal((D, D), dtype=np.float32) * 0.02).astype(np.float32)
    temp = np.ones((H, 1), dtype=np.float32)
    W_out = (rng.standard_normal((D, D), dtype=np.float32) * 0.02).astype(np.float32)
    b_out = np.zeros((D,), dtype=np.float32)
    y = kernel(x=x, W_qkv=W_qkv, temp=temp, W_out=W_out, b_out=b_out)
    print("kernel ran, y shape", y.shape, "mean abs", np.abs(y).mean())


# revision 51
# speedup vs baseline: 1.1939x; 1.0002x over previous
"""AttentionTSSA Trainium2 kernel (v2).

Sharding: data-parallel over batch. B=8 -> one batch element per NeuronCore,
zero collectives. Host slices inputs / stacks outputs.

Per-core math (x: [N=4096, D=1024], heads h=16, head dim d=64):
  w[c, n]   = (x @ W_qkv.T).T                 (c = h*64+dd, channel-major)
  s[c]      = sum_n w^2
  logits[h,n] = sum_dd w^2[c,n] * temp[h]/max(s[c],eps)
  Pi        = softmax_h(logits)
  dots[c]   = (sum_n Pi[h(c),n] * w^2[c,n]) / (sum_n Pi[h(c),n] + 1e-8)
  u         = w * Pi_bcast          (overwrites w in place)
  y         = u.T @ (-1/(1+dots) * W_out.T) + b_out

Layout/engine plan (local HW exec ~345us vs ~634us for the v1 kernel):
  - x.T, W_qkv.T, W_out.T shipped bf16 from host (x pre-transposed on
    the host), so every input load is a plain direct2d DMA — no device
    transposes at all. All MM in bf16 at full PE rate.
  - DMA ordering rule (measured): same-queue, same-type DMAs pipeline at
    ~1.3us cadence; every type (direct2d <-> transpose) or queue switch
    costs a ~3us serializing handoff -> batch by type, one switch each.
  - phase A (PE-bound, 99% busy): MM1; PSUM evicted twice: ACT copy -> w
    (bf16) and DVE STT -> w^2 (bf16) + s accumulator. Stats batched into
    [128, NT]-wide ops; per-head scalars broadcast via tiny sel matmuls.
  - phase D (DVE-bound): per 512-chunk: logits matmul (PE) -> Exp (ACT,
    the only act-table func used) -> sum_h (PE) -> f32r evict (ACT) ->
    bcast (PE) -> reciprocal_approx_fast + Pi STT (DVE); per (tile, 1024-
    chunk): Pi bcast matmul (PE) + ACT evict + DVE dots-STT + DVE u-mult
    (u overwrites w in place; w^2 stays pristine). Last block runs dots
    before u so attn -> wob -> MM2 unblocks early.
  - MM2 (PE-bound, 99% busy): pure PE accumulation, DVE STT eviction with
    fused bias add, per-128-row gpsimd-issued output DMAs.
"""

import sys

sys.path.insert(0, "/opt/trn_rl_repo")

import numpy as np
import concourse.bacc as bacc
import concourse.tile as tile
from concourse import mybir
from concourse.bass_utils import run_bass_kernel_spmd

F32 = mybir.dt.float32
F32R = mybir.dt.float32r
BF16 = mybir.dt.bfloat16
MUL = mybir.AluOpType.mult
ADD = mybir.AluOpType.add
EXP = mybir.ActivationFunctionType.Exp

B, N, D = 8, 4096, 1024
H, HD = 16, 64
P = 128
NT = D // P          # 8 col-partition tiles
CH = 512             # n-chunk for MM work
NCH = N // CH        # 8 chunks
QN = 1024            # n-quarter for xT DMA transposes
NQ = N // QN         # 4 quarters


def build():
    nc = bacc.Bacc()
    x_t = nc.dram_tensor("xTbf", [D, N], BF16, kind="ExternalInput")   # x.T
    wq_t = nc.dram_tensor("wqT", [D, D], BF16, kind="ExternalInput")     # W_qkv.T
    wo_t = nc.dram_tensor("woT", [D, D], BF16, kind="ExternalInput")     # W_out.T
    temp_t = nc.dram_tensor("temp", [H, 1], F32, kind="ExternalInput")
    sel_t = nc.dram_tensor("sel", [NT, H, P], F32, kind="ExternalInput")
    selb_t = nc.dram_tensor("selb", [NT, H, P], BF16, kind="ExternalInput")
    selT_t = nc.dram_tensor("selT", [NT, P, H], F32, kind="ExternalInput")
    bias_t = nc.dram_tensor("bout", [1, D], F32R, kind="ExternalInput")
    y_t = nc.dram_tensor("y", [N, D], F32, kind="ExternalOutput")

    with tile.TileContext(nc) as tc:
        with (
            tc.tile_pool(name="consts", bufs=1) as consts,
            tc.tile_pool(name="wmat", bufs=1) as wmat,
            tc.tile_pool(name="wsb", bufs=1) as wsb,
            tc.tile_pool(name="small", bufs=1) as small,
        ):
            # ---------- weights first (gate MM1 start), then constants ----
            # persistent big tensors: w (becomes u in place), w^2
            w_tiles = [wsb.tile([P, N], BF16, tag=f"w{t}", name=f"w{t}") for t in range(NT)]
            w2_tiles = [wsb.tile([P, N], BF16, tag=f"w2_{t}", name=f"w2_{t}") for t in range(NT)]
            s_all = small.tile([P, NT * NCH], F32, tag="s_all")
            d_all = small.tile([P, NT * (N // 1024)], F32, tag="d_all")
            sumpi_c = small.tile([H, NCH], F32, tag="sumpi_c")
            bias_sb = small.tile([P, D], F32, tag="bias_sb")

            # ---------- phase A: xT by DMA transpose; w = x @ WqkvT ----------
            with (
                tc.tile_pool(name="xq", bufs=2) as xqp,
                tc.tile_pool(name="scrA", bufs=2) as scrA,
                tc.tile_pool(name="psA", bufs=3, space="PSUM") as psA,
                tc.tile_pool(name="psBb", bufs=1, space="PSUM") as psBb,
            ):
                # q0 transposes FIRST in program order: the first DMA issued
                # gets a conservative cross-queue ordering wait, and nothing
                # may gate the transposes that in turn gate MM1
                # DMA ordering rules learned from traces: same-queue, same-
                # TYPE DMAs pipeline at full rate; every transition between
                # DMA types (direct2d <-> xbar transpose) or queues costs a
                # ~3us serializing semaphore handoff. So: one direct2d batch
                # (wq, gates MM1), one transpose batch (q0+q1), then the
                # small constants (needed only ~100us in), then q2/q3.
                wq_sb = wmat.tile([P, NT, D], BF16, tag="wm")
                for k in range(NT):
                    nc.sync.dma_start(
                        out=wq_sb[:, k, :], in_=wq_t[k * P : (k + 1) * P, :]
                    )
                xq01 = []
                for q in range(2):
                    xq = xqp.tile([P, NT, QN], BF16, tag="xq")
                    xq01.append(xq)
                    for k in range(NT):
                        nc.sync.dma_start(
                            out=xq[:, k, :],
                            in_=x_t[k * P : (k + 1) * P, q * QN : (q + 1) * QN],
                        )
                temp_sb = consts.tile([H, 1], F32)
                nc.sync.dma_start(out=temp_sb, in_=temp_t[:, :])
                bias_r = consts.tile([1, D], F32R)
                nc.sync.dma_start(out=bias_r, in_=bias_t[:, :])
                sel_sb = consts.tile([H, NT, P], F32)
                nc.sync.dma_start(out=sel_sb, in_=sel_t.rearrange("t h p -> h t p"))
                selb_sb = consts.tile([H, NT, P], BF16)
                nc.sync.dma_start(out=selb_sb, in_=selb_t.rearrange("t h p -> h t p"))
                selT_sb = consts.tile([P, NT, H], F32)
                nc.sync.dma_start(out=selT_sb, in_=selT_t.rearrange("t p h -> p t h"))
                ones16_f = consts.tile([H, 1], F32)
                nc.vector.memset(ones16_f, 1.0)
                ones16_r = consts.tile([H, 1], F32R)
                nc.vector.tensor_copy(ones16_r, ones16_f)
                ones1x16_f = consts.tile([1, H], F32)
                nc.vector.memset(ones1x16_f, 1.0)
                ones1x16_r = consts.tile([1, H], F32R)
                nc.vector.tensor_copy(ones1x16_r, ones1x16_f)
                ones1x128_f = consts.tile([1, P], F32)
                nc.vector.memset(ones1x128_f, 1.0)
                ones1x128_r = consts.tile([1, P], F32R)
                nc.vector.tensor_copy(ones1x128_r, ones1x128_f)

                for q in range(NQ):
                    if q < 2:
                        xq = xq01[q]
                    else:
                        xq = xqp.tile([P, NT, QN], BF16, tag="xq")
                        for k in range(NT):
                            nc.sync.dma_start(
                                out=xq[:, k, :],
                                in_=x_t[k * P : (k + 1) * P, q * QN : (q + 1) * QN],
                            )
                    for c2 in range(QN // CH):
                        c = q * (QN // CH) + c2
                        cs = slice(c * CH, (c + 1) * CH)
                        for t in range(NT):
                            w_ps = psA.tile([P, CH], F32, tag="mm1")
                            for k in range(NT):
                                nc.tensor.matmul(
                                    w_ps,
                                    wq_sb[:, k, t * P : (t + 1) * P],
                                    xq[:, k, c2 * CH : (c2 + 1) * CH],
                                    start=(k == 0),
                                    stop=(k == NT - 1),
                                )
                            nc.scalar.copy(out=w_tiles[t][:, cs], in_=w_ps)
                            nc.vector.scalar_tensor_tensor(
                                out=w2_tiles[t][:, cs],
                                in0=w_tiles[t][:, cs],
                                scalar=1.0,
                                in1=w_tiles[t][:, cs],
                                op0=MUL,
                                op1=MUL,
                                accum_out=s_all[:, t * NCH + c : t * NCH + c + 1],
                            )

                # bias broadcast [128, D] via 1-row matmuls; emitted last so
                # the in-order PE queue isn't head-of-line blocked waiting
                # for the bias DMA at startup
                bb_ps = psBb.tile([P, D], F32, tag="bb")
                for oh in range(2):
                    os_ = slice(oh * CH, (oh + 1) * CH)
                    nc.tensor.matmul(
                        bb_ps[:, os_], ones1x128_r, bias_r[:, os_],
                        start=True, stop=True,
                    )
                nc.scalar.copy(out=bias_sb, in_=bb_ps)

            # W_out.T load (reuses wq's slot; overlaps phase D)
            wo_sb = wmat.tile([P, NT, D], BF16, tag="wm")
            for k in range(NT):
                nc.sync.dma_start(out=wo_sb[:, k, :], in_=wo_t[k * P : (k + 1) * P, :])

            # ---------- stats 1: lbig[t][c, h] = sel*temp[h]/max(s,eps) ----
            with tc.tile_pool(name="psS1", bufs=1, space="PSUM") as psS1:
                tb_ps = psS1.tile([P, NT], F32, tag="tb")
                for t in range(NT):
                    nc.tensor.matmul(
                        tb_ps[:, t : t + 1], sel_sb[:, t, :], temp_sb,
                        start=True, stop=True,
                    )
                s_red = small.tile([P, NT], F32, tag="s_red")
                nc.vector.reduce_sum(
                    s_red,
                    s_all.rearrange("p (t c) -> p t c", c=NCH),
                    axis=mybir.AxisListType.X,
                )
                nc.vector.tensor_scalar_max(out=s_red, in0=s_red, scalar1=1e-24)
                rcp = small.tile([P, NT], F32, tag="rcp")
                nc.vector.reciprocal(rcp, s_red)
                inv_all = small.tile([P, NT], F32, tag="inv_all")
                nc.vector.tensor_mul(inv_all, rcp, tb_ps)
                lbig = small.tile([P, NT, H], BF16, tag="lbig")
                for t in range(NT):
                    nc.vector.tensor_scalar_mul(
                        out=lbig[:, t, :],
                        in0=selT_sb[:, t, :],
                        scalar1=inv_all[:, t : t + 1],
                    )

            # ---------- phase D: logits -> softmax -> u, dots (pipelined) --
            # logits/softmax work per 512-chunk (PSUM-bank sized); the bulk
            # pib/u/dots passes run 1024-wide to halve DVE/ACT op count.
            pi_bf = small.tile([H, N], BF16, tag="pi_bf")
            CHD = 1024
            NCHD = N // CHD
            with (
                tc.tile_pool(name="scrD", bufs=3) as scrD,
                tc.tile_pool(name="psL", bufs=2, space="PSUM") as psL,
                tc.tile_pool(name="psS", bufs=1, space="PSUM") as psS,
                tc.tile_pool(name="psP", bufs=2, space="PSUM") as psP,
            ):
                for cd in range(NCHD):
                    for hh in range(CHD // CH):
                        c = cd * (CHD // CH) + hh
                        cs = slice(c * CH, (c + 1) * CH)
                        lg_ps = psL.tile([H, CH], F32, tag="lg")
                        for t in range(NT):
                            nc.tensor.matmul(
                                lg_ps, lbig[:, t, :], w2_tiles[t][:, cs],
                                start=(t == 0), stop=(t == NT - 1),
                            )
                        e_sb = scrD.tile([H, CH], F32R, tag="e_sb")
                        nc.scalar.activation(out=e_sb, in_=lg_ps, func=EXP)
                        se_ps = psS.tile([1, CH], F32, tag="se")
                        nc.tensor.matmul(
                            se_ps, ones16_r, e_sb, start=True, stop=True
                        )
                        ses = scrD.tile([1, CH], F32R, tag="ses", bufs=2)
                        nc.scalar.copy(out=ses, in_=se_ps)
                        rb_ps = psS.tile([H, CH], F32, tag="rb")
                        nc.tensor.matmul(
                            rb_ps, ones1x16_r, ses, start=True, stop=True
                        )
                        rcb = scrD.tile([H, CH], F32, tag="rcb", bufs=2)
                        nc.vector.reciprocal_approx_fast(out=rcb, in_=rb_ps)
                        nc.vector.scalar_tensor_tensor(
                            out=pi_bf[:, cs],
                            in0=e_sb.bitcast(F32),
                            scalar=1.0,
                            in1=rcb,
                            op0=MUL,
                            op1=MUL,
                            accum_out=sumpi_c[:, c : c + 1],
                        )
                    ds = slice(cd * CHD, (cd + 1) * CHD)
                    # In the last block, run all dots accumulations before
                    # the u-mults: dots gates attn -> wob -> ALL of MM2,
                    # while the last u-mults only gate MM2's final n-tiles
                    # and hide under its first matmuls.
                    last = cd == NCHD - 1
                    deferred_u = []
                    for t in range(NT):
                        pib_ps = psP.tile([P, CHD], F32, tag="pib")
                        for hh in range(CHD // CH):
                            nc.tensor.matmul(
                                pib_ps[:, hh * CH : (hh + 1) * CH],
                                selb_sb[:, t, :],
                                pi_bf[:, cd * CHD + hh * CH : cd * CHD + (hh + 1) * CH],
                                start=True,
                                stop=True,
                            )
                        if last:
                            pib_sb = scrD.tile(
                                [P, CHD], BF16, tag="pib_last", bufs=NT
                            )
                        else:
                            pib_sb = scrD.tile(
                                [P, CHD], BF16, tag="pib_sb", bufs=2
                            )
                        nc.scalar.copy(out=pib_sb, in_=pib_ps)
                        junk = scrD.tile([P, CHD], BF16, tag="junkD", bufs=1)
                        nc.vector.scalar_tensor_tensor(
                            out=junk,
                            in0=pib_sb,
                            scalar=1.0,
                            in1=w2_tiles[t][:, ds],
                            op0=MUL,
                            op1=MUL,
                            accum_out=d_all[:, t * NCHD + cd : t * NCHD + cd + 1],
                        )

                        if last:
                            deferred_u.append((t, pib_sb))
                        else:
                            # u overwrites w in place (w^2 stays pristine)
                            nc.vector.tensor_mul(
                                w_tiles[t][:, ds], w_tiles[t][:, ds], pib_sb
                            )
                    for t, pib_sb in deferred_u:
                        nc.vector.tensor_mul(
                            w_tiles[t][:, ds], w_tiles[t][:, ds], pib_sb
                        )

            # ---------- stats 2: attn; wob = -attn * WoutT (bf16) ----------
            wob, _wob_free = tc.tile([P, NT, D], BF16, name="wob")
            with tc.tile_pool(name="psS2", bufs=1, space="PSUM") as psS2:
                sumpi = small.tile([H, 1], F32, tag="sumpi")
                nc.vector.reduce_sum(sumpi, sumpi_c, axis=mybir.AxisListType.X)
                nc.vector.tensor_scalar_add(out=sumpi, in0=sumpi, scalar1=1e-8)
                ispi = small.tile([H, 1], F32, tag="ispi")
                nc.vector.reciprocal(ispi, sumpi)
                isp_ps = psS2.tile([P, NT], F32, tag="isp")
                for t in range(NT):
                    nc.tensor.matmul(
                        isp_ps[:, t : t + 1], sel_sb[:, t, :], ispi,
                        start=True, stop=True,
                    )
                d_red = small.tile([P, NT], F32, tag="d_red")
                nc.vector.reduce_sum(
                    d_red,
                    d_all.rearrange("p (t c) -> p t c", c=N // 1024),
                    axis=mybir.AxisListType.X,
                )
                attn_neg = small.tile([P, NT], F32, tag="attn_neg")
                nc.vector.tensor_mul(attn_neg, d_red, isp_ps)
                nc.vector.tensor_scalar_add(out=attn_neg, in0=attn_neg, scalar1=1.0)
                nc.vector.reciprocal(attn_neg, attn_neg)
                nc.vector.tensor_scalar_mul(
                    out=attn_neg, in0=attn_neg, scalar1=-1.0
                )
                for t in range(NT):
                    if t % 2 == 0:
                        nc.scalar.mul(
                            out=wob[:, t, :],
                            in_=wo_sb[:, t, :],
                            mul=attn_neg[:, t : t + 1],
                        )
                    else:
                        nc.vector.tensor_scalar_mul(
                            out=wob[:, t, :],
                            in0=wo_sb[:, t, :],
                            scalar1=attn_neg[:, t : t + 1],
                        )

            # ---------- MM2: y = u.T @ wob + b ----------
            with (
                tc.tile_pool(name="och", bufs=3) as och,
                tc.tile_pool(name="psMM2", bufs=4, space="PSUM") as psMM2,
            ):
                MS = CH // P  # 4 n-subtiles per 512-chunk
                for c in range(NCH):
                    for m in range(MS):
                        ms_ = slice(c * CH + m * P, c * CH + (m + 1) * P)
                        outf = och.tile([P, D], F32, tag="outf")
                        for oh in range(2):
                            os_ = slice(oh * CH, (oh + 1) * CH)
                            f_ps = psMM2.tile([P, CH], F32, tag="mm2")
                            for t in range(NT):
                                nc.tensor.matmul(
                                    f_ps,
                                    w_tiles[t][:, ms_],
                                    wob[:, t, os_],
                                    start=(t == 0),
                                    stop=(t == NT - 1),
                                )
                            nc.vector.scalar_tensor_tensor(
                                out=outf[:, os_],
                                in0=f_ps,
                                scalar=1.0,
                                in1=bias_sb[:, os_],
                                op0=MUL,
                                op1=ADD,
                            )
                        nc.gpsimd.dma_start(out=y_t[ms_, :], in_=outf)
            _wob_free()

    if not nc.is_finalized():
        nc.finalize()
    return nc


_NC_CACHE = None
_LAST_IN_MAPS = None
_RUNNER = None


def _make_runner(nc, n_cores):
    """Like bass2jax.run_bass_via_pjrt but with the jitted callable cached,
    so repeat calls don't re-trace/re-compile the XLA wrapper."""
    import jax
    from jax.experimental.shard_map import shard_map
    from jax.sharding import Mesh, PartitionSpec
    from concourse import mybir as _mybir
    from concourse.bass2jax import (
        _bass_exec_p,
        install_neuronx_cc_hook,
        partition_id_tensor,
    )

    install_neuronx_cc_hook()

    partition_name = nc.partition_id_tensor.name if nc.partition_id_tensor else None
    in_names, out_names, out_avals, zero_outs = [], [], [], []
    for alloc in nc.m.functions[0].allocations:
        if not isinstance(alloc, _mybir.MemoryLocationSet):
            continue
        name = alloc.memorylocations[0].name
        if alloc.kind == "ExternalInput":
            if name != partition_name:
                in_names.append(name)
        elif alloc.kind == "ExternalOutput":
            shape = tuple(alloc.tensor_shape)
            dtype = _mybir.dt.np(alloc.dtype)
            out_names.append(name)
            out_avals.append(jax.core.ShapedArray(shape, dtype))
            zero_outs.append(np.zeros(shape, dtype))
    n_params = len(in_names)
    n_outs = len(out_names)
    all_in_names = in_names + out_names + (
        [partition_name] if partition_name else []
    )
    donate = tuple(range(n_params, n_params + n_outs))

    def _body(*args):
        operands = list(args)
        if partition_name is not None:
            operands.append(partition_id_tensor())
        outs = _bass_exec_p.bind(
            *operands,
            out_avals=tuple(out_avals),
            in_names=tuple(all_in_names),
            out_names=tuple(out_names),
            lowering_input_output_aliases=(),
            sim_require_finite=True,
            sim_require_nnan=True,
            nc=nc,
        )
        return tuple(outs)

    devices = jax.devices()[:n_cores]
    mesh = Mesh(np.asarray(devices), ("core",))
    in_specs = (PartitionSpec("core"),) * (n_params + n_outs)
    out_specs = (PartitionSpec("core"),) * n_outs
    sharded = jax.jit(
        shard_map(
            _body, mesh=mesh, in_specs=in_specs, out_specs=out_specs, check_rep=False
        ),
        donate_argnums=donate,
        keep_unused=True,
    )

    def run(in_maps):
        concat_in = [
            np.concatenate([np.asarray(m[name]) for m in in_maps], axis=0)
            for name in in_names
        ]
        concat_zeros = [
            np.zeros((n_cores * z.shape[0], *z.shape[1:]), z.dtype)
            for z in zero_outs
        ]
        out_arrs = sharded(*concat_in, *concat_zeros)
        return {
            name: np.asarray(out_arrs[i]).reshape(n_cores, *out_avals[i].shape)
            for i, name in enumerate(out_names)
        }

    run.sharded = sharded
    run.meta = (in_names, out_names, out_avals, n_params, n_outs)
    return run


def kernel(x, W_qkv, temp, W_out, b_out):
    global _NC_CACHE, _RUNNER
    if _NC_CACHE is None:
        _NC_CACHE = build()
        _RUNNER = _make_runner(_NC_CACHE, B)

    import ml_dtypes

    bf16 = ml_dtypes.bfloat16
    x = np.asarray(x, dtype=np.float32)
    xbf = x.astype(bf16)
    wqT = np.ascontiguousarray(np.asarray(W_qkv, dtype=np.float32).T).astype(bf16)
    woT = np.ascontiguousarray(np.asarray(W_out, dtype=np.float32).T).astype(bf16)
    temp = np.ascontiguousarray(np.asarray(temp, dtype=np.float32).reshape(H, 1))
    bout = np.ascontiguousarray(np.asarray(b_out, dtype=np.float32).reshape(1, D))

    sel = np.zeros((NT, H, P), dtype=np.float32)
    for t in range(NT):
        sel[t, 2 * t, 0:HD] = 1.0
        sel[t, 2 * t + 1, HD:P] = 1.0
    selT = np.ascontiguousarray(sel.transpose(0, 2, 1))

    in_maps = [
        {"xTbf": np.ascontiguousarray(xbf[i].T), "wqT": wqT, "woT": woT,
         "temp": temp, "bout": bout, "sel": sel, "selb": sel.astype(bf16),
         "selT": selT}
        for i in range(B)
    ]
    global _LAST_IN_MAPS
    _LAST_IN_MAPS = in_maps
    out = _RUNNER(in_maps)
    return out["y"]


if __name__ == "__main__":
    rng = np.random.default_rng(0)
    x = rng.standard_normal((B, N, D), dtype=np.float32)
    W_qkv = (rng.standard_normal((D, D), dtype=np.float32) * 0.02).astype(np.float32)
    temp = np.ones((H, 1), dtype=np.float32)
    W_out = (rng.standard_normal((D, D), dtype=np.float32) * 0.02).astype(np.float32)
    b_out = np.zeros((D,), dtype=np.float32)
    y = kernel(x=x, W_qkv=W_qkv, temp=temp, W_out=W_out, b_out=b_out)
    print("kernel ran, y shape", y.shape, "mean abs", np.abs(y).mean())
